# revision 13
# baseline (speedup 1.0000x reference)
"""CLIP attention block (LN(attn(x) @ W_out)) on 8 TRN2 NeuronCores.

Problem (hardcoded): x [4, 2048, 1024] f32, mask [4, 2048] bool,
w_qkv [1024, 3072], w_out [1024, 1024], ln_g [1024].
16 heads x 64 dim, causal, scale = 1/8. Output [4, 2048, 1024] f32.

Sharding: core = (batch b, parity rho). Each core computes the final
output rows for queries g of batch b with g % 2 == rho (1024 tokens).
Interleaving queries by parity makes the causal workload identical on
every core (SPMD-uniform static program): local query block i (128 rows,
globals 256*i + 2*r + rho) attends keys < 256*(i+1), so block i needs
exactly 2*(i+1) key-chunks of 128 regardless of rho; only the diagonal
mask tile differs per core, and that is input data.

Per-core plan (all matmuls in float32r: 1 cyc/row at N>=256, ~1.5e-4 rel):
  QT[inner, 1024] = (Wq^T chunks) @ xqT        (Q pre-scaled by 1/8)
  KT[inner, 2048], V[tok, inner] from xfT; V stored augmented per head
  as [V_h | m] (m = key padding multiplier) so the AV matmul's 65th
  output row accumulates the softmax denominator for free.
  S^T[k, q] = KT-slice^T @ QT-slice  (k on partitions -> exp needs no
  transposes anywhere; softmax uses no max-subtraction: |S| <~ 10).
  P^T = exp(S^T + causal mask), O^T[65, q] = sum_kc Vaug^T @ P^T.
  O^T rows 0..63 scaled by 1/denom (row 64): reciprocal + gpsimd
  partition_broadcast + DVE multiply, into OT[inner, 1024].
  Z[tok, dim] = sum_hp OT-slice^T @ w_out chunk; layernorm * gamma; out.
K^T/Q^T/O^T stream through DRAM scratch so SBUF holds only V resident;
DMAs are consolidated into ~100 large transfers (HWDGE issue is ~650ns
per dma_start and was the v1 bottleneck).
"""

import threading

import numpy as np

import concourse.bass as bass
import concourse.mybir as mybir
import concourse.tile as tile
from concourse import bacc
import concourse.bass_utils as bass_utils

F32 = mybir.dt.float32
F32R = mybir.dt.float32r

B, N, DIM = 4, 2048, 1024
HEADS, DH = 16, 64
INNER = HEADS * DH          # 1024
SCALE = DH ** -0.5          # 0.125
LOC = N // 2                # 1024 local query tokens per core
NEG = -1.0e30
EPS = 1e-5

NC = 8                      # cores
HP = HEADS // 2             # 8 head pairs
KC = N // 128               # 16 key chunks
G = LOC // 256              # 4 q-groups of 256


def build(reps=1):
    nc = bacc.Bacc("TRN2", target_bir_lowering=False, debug=False, num_devices=NC)

    xfT = nc.dram_tensor("xfT", [DIM, N], F32R, kind="ExternalInput").ap()
    xqT = nc.dram_tensor("xqT", [DIM, LOC], F32R, kind="ExternalInput").ap()
    wq = nc.dram_tensor("wq", [DIM, INNER], F32R, kind="ExternalInput").ap()
    wk = nc.dram_tensor("wk", [DIM, INNER], F32R, kind="ExternalInput").ap()
    wv = nc.dram_tensor("wv", [DIM, INNER], F32R, kind="ExternalInput").ap()
    wout = nc.dram_tensor("wout", [INNER, DIM], F32R, kind="ExternalInput").ap()
    lng = nc.dram_tensor("lng", [1, DIM], F32, kind="ExternalInput").ap()
    dmask = nc.dram_tensor("dmask", [128, 1024], F32, kind="ExternalInput").ap()
    mvecT = nc.dram_tensor("mvecT", [128, KC], F32, kind="ExternalInput").ap()
    out = nc.dram_tensor("out", [LOC, DIM], F32, kind="ExternalOutput").ap()

    with nc.allow_low_precision(reason="float32r matmul staging"), \
         tile.TileContext(nc) as tc:
        for _ in range(reps):
            _build_body(nc, tc, xfT, xqT, wq, wk, wv, wout, lng, dmask, mvecT, out)

    nc.compile()
    return nc


def _build_body(nc, tc, xfT, xqT, wq, wk, wv, wout, lng, dmask, mvecT, out):
    mm = nc.tensor.matmul
    A = mybir.ActivationFunctionType

    # ---------------- prep: small residents ----------------
    res = tc.alloc_tile_pool(name="res", bufs=1)
    DM = res.tile([128, 1024], F32, tag="DM")
    nc.sync.dma_start(DM[:], dmask[:])
    MV = res.tile([128, KC], F32, tag="MV")
    nc.sync.dma_start(MV[:], mvecT[:])
    ones16 = res.tile([128, HEADS, 1], F32, tag="ones16")
    nc.vector.memset(ones16[:], 1.0)
    grow = res.tile([1, DIM], F32, tag="grow")
    nc.sync.dma_start(grow[:], lng[:])
    GB = res.tile([128, DIM], F32, tag="GB")
    nc.gpsimd.partition_broadcast(GB[:], grow[:])
    epst = res.tile([128, 1], F32, tag="epst")
    nc.vector.memset(epst[:], EPS)

    # V: 16 token-chunk tiles, per head [64 V cols | padding-multiplier col]
    vpool = tc.alloc_tile_pool(name="vres", bufs=1)
    V = [vpool.tile([128, HEADS, DH + 1], F32R, tag=f"V{i}", name=f"V{i}")
         for i in range(KC)]

    # DRAM scratch for K^T, Q^T, O^T (streamed back per head-pair)
    dram = tc.alloc_tile_pool(name="dram", bufs=1, space="DRAM")
    KTd = [dram.tile([128, N], F32R, tag=f"ktd{hp}", name=f"ktd{hp}")
           for hp in range(HP)]
    QTd = [dram.tile([128, LOC], F32R, tag=f"qtd{hp}", name=f"qtd{hp}")
           for hp in range(HP)]
    OTd = [dram.tile([128, LOC], F32R, tag=f"otd{hp}", name=f"otd{hp}")
           for hp in range(HP)]

    pps = tc.alloc_tile_pool(name="pps", bufs=2, space="PSUM")

    # ---------------- phase V: V projection ----------------
    xf_pool = tc.alloc_tile_pool(name="xf", bufs=1)
    XF = [xf_pool.tile([128, N], F32R, tag=f"xf{dc}", name=f"xf{dc}")
          for dc in range(8)]
    for dc in range(8):
        nc.sync.dma_start(XF[dc][:], xfT[dc * 128:(dc + 1) * 128, :])

    wv_pool = tc.alloc_tile_pool(name="wv", bufs=1)
    WV = [wv_pool.tile([128, INNER], F32R, tag=f"wv{dc}", name=f"wv{dc}")
          for dc in range(8)]
    for dc in range(8):
        nc.sync.dma_start(WV[dc][:], wv[dc * 128:(dc + 1) * 128, :])

    for tci in range(KC):
        for ig in range(2):
            vp = pps.tile([128, 512], F32, tag="pp", name="vp")
            for dc in range(8):
                mm(vp[:], XF[dc][:, tci * 128:(tci + 1) * 128],
                   WV[dc][:, ig * 512:(ig + 1) * 512],
                   start=(dc == 0), stop=(dc == 7))
            dst = V[tci][:, ig * 8:(ig + 1) * 8, 0:DH]
            nc.vector.tensor_scalar_mul(
                dst, vp[:].rearrange("p (h d) -> p h d", d=DH),
                MV[:, tci:tci + 1])
        nc.vector.tensor_scalar_mul(
            V[tci][:, :, DH:DH + 1], ones16[:], MV[:, tci:tci + 1])
    wv_pool.release()

    # ---------------- phase K: K^T projection -> DRAM ----------------
    # wk column-block for head-pair hp, all 8 dim-chunks in one DMA:
    # WKC[p, dc, j] = wk[dc*128 + p, hp*128 + j]
    wk_r = wk.rearrange("(dc p) j -> p dc j", p=128)
    wq_r = wq.rearrange("(dc p) j -> p dc j", p=128)
    wkc_pool = tc.alloc_tile_pool(name="wkc", bufs=2)
    kst_pool = tc.alloc_tile_pool(name="kst", bufs=2)
    for hp in range(HP):
        wkc = wkc_pool.tile([128, 8, 128], F32R, tag="wkc", name="wkc")
        nc.sync.dma_start(wkc[:], wk_r[:, :, hp * 128:(hp + 1) * 128])
        kst = kst_pool.tile([128, N], F32R, tag="kst", name="kst")
        for tg in range(4):
            kp = pps.tile([128, 512], F32, tag="pp", name="kp")
            for dc in range(8):
                mm(kp[:], wkc[:, dc, :], XF[dc][:, tg * 512:(tg + 1) * 512],
                   start=(dc == 0), stop=(dc == 7))
            nc.vector.tensor_copy(kst[:, tg * 512:(tg + 1) * 512], kp[:])
        nc.gpsimd.dma_start(KTd[hp][:], kst[:])
    kst_pool.release()
    wkc_pool.release()
    xf_pool.release()

    # ---------------- phase Q: Q^T projection (pre-scaled) -> DRAM --------
    xq_pool = tc.alloc_tile_pool(name="xq", bufs=1)
    XQ = [xq_pool.tile([128, LOC], F32R, tag=f"xq{dc}", name=f"xq{dc}")
          for dc in range(8)]
    for dc in range(8):
        nc.sync.dma_start(XQ[dc][:], xqT[dc * 128:(dc + 1) * 128, :])
    wqc_pool = tc.alloc_tile_pool(name="wqc", bufs=2)
    qst_pool = tc.alloc_tile_pool(name="qst", bufs=2)
    for hp in range(HP):
        wqc = wqc_pool.tile([128, 8, 128], F32R, tag="wqc", name="wqc")
        nc.sync.dma_start(wqc[:], wq_r[:, :, hp * 128:(hp + 1) * 128])
        qst = qst_pool.tile([128, LOC], F32R, tag="qst", name="qst")
        for tg in range(2):
            qp = pps.tile([128, 512], F32, tag="pp", name="qp")
            for dc in range(8):
                mm(qp[:], wqc[:, dc, :], XQ[dc][:, tg * 512:(tg + 1) * 512],
                   start=(dc == 0), stop=(dc == 7))
            nc.vector.tensor_scalar_mul(qst[:, tg * 512:(tg + 1) * 512],
                                        qp[:], SCALE)
        nc.gpsimd.dma_start(QTd[hp][:], qst[:])
    qst_pool.release()
    wqc_pool.release()
    xq_pool.release()
    pps.release()

    # ---------------- attention ----------------
    ktp_pool = tc.alloc_tile_pool(name="ktp", bufs=2)
    qtp_pool = tc.alloc_tile_pool(name="qtp", bufs=2)
    ots_pool = tc.alloc_tile_pool(name="ots", bufs=2)
    pt_pool = tc.alloc_tile_pool(name="pt", bufs=3)
    rc_pool = tc.alloc_tile_pool(name="rc", bufs=2)
    st_ps = tc.alloc_tile_pool(name="stps", bufs=2, space="PSUM")
    o_ps = tc.alloc_tile_pool(name="ops", bufs=2, space="PSUM")

    for hp in range(HP):
        KTt = ktp_pool.tile([128, N], F32R, tag="ktt", name="KTt")
        nc.sync.dma_start(KTt[:], KTd[hp][:])
        QTt = qtp_pool.tile([128, LOC], F32R, tag="qtt", name="QTt")
        nc.sync.dma_start(QTt[:], QTd[hp][:])
        OTt = ots_pool.tile([128, LOC], F32R, tag="ott", name="OTt")
        for h2 in range(2):
            h = 2 * hp + h2
            for g in range(G):
                n_kc = 4 * (g + 1)
                op = o_ps.tile([DH + 1, 256], F32, tag="o", name="op")
                for s in range(g + 1):
                    st = st_ps.tile([128, 1024], F32, tag="st", name="st")
                    for j in range(4):
                        kc = 4 * s + j
                        mm(st[:, j * 256:(j + 1) * 256],
                           KTt[h2 * DH:(h2 + 1) * DH, kc * 128:(kc + 1) * 128],
                           QTt[h2 * DH:(h2 + 1) * DH, g * 256:(g + 1) * 256],
                           start=True, stop=True)
                    if s == g:  # diagonal strip: one fused causal-mask add
                        nc.vector.tensor_add(st[:], st[:], DM[:])
                    pt = pt_pool.tile([128, 1024], F32R, tag="pt", name="pt")
                    nc.scalar.activation(pt[:], st[:], A.Exp)
                    for j in range(4):
                        kc = 4 * s + j
                        mm(op[:], V[kc][:, h, :], pt[:, j * 256:(j + 1) * 256],
                           start=(kc == 0), stop=(kc == n_kc - 1))
                rcp = rc_pool.tile([1, 256], F32, tag="rcp", name="rcp")
                nc.vector.reciprocal(rcp[:], op[DH:DH + 1, :])
                rbs = rc_pool.tile([DH, 256], F32, tag="rbs", name="rbs")
                nc.gpsimd.partition_broadcast(rbs[:], rcp[:])
                nc.vector.tensor_mul(
                    OTt[h2 * DH:(h2 + 1) * DH, g * 256:(g + 1) * 256],
                    op[0:DH, :], rbs[:])
        nc.gpsimd.dma_start(OTd[hp][:], OTt[:])

    rc_pool.release()
    pt_pool.release()
    ots_pool.release()
    qtp_pool.release()
    ktp_pool.release()
    o_ps.release()
    st_ps.release()

    # ---------------- out projection + layernorm ----------------
    wo_pool = tc.alloc_tile_pool(name="wo", bufs=1)
    WO = [wo_pool.tile([128, DIM], F32R, tag=f"wo{hp}", name=f"wo{hp}")
          for hp in range(HP)]
    otz_pool = tc.alloc_tile_pool(name="otz", bufs=1)
    OTZ = [otz_pool.tile([128, LOC], F32R, tag=f"otz{hp}", name=f"otz{hp}")
           for hp in range(HP)]
    for hp in range(HP):
        nc.sync.dma_start(WO[hp][:], wout[hp * 128:(hp + 1) * 128, :])
        nc.sync.dma_start(OTZ[hp][:], OTd[hp][:])
    stat_pool = tc.alloc_tile_pool(name="stat", bufs=2)
    stage_pool = tc.alloc_tile_pool(name="stage", bufs=2)
    z_ps = tc.alloc_tile_pool(name="zps", bufs=1, space="PSUM")

    for tbg in range(2):
        zps = {}
        for ti in range(4):
            for half in range(2):
                zps[(ti, half)] = z_ps.tile([128, 512], F32, tag=f"z{ti}{half}",
                                            name=f"z{ti}{half}")
        for hp in range(HP):
            for ti in range(4):
                tb = tbg * 4 + ti
                for half in range(2):
                    mm(zps[(ti, half)][:],
                       OTZ[hp][:, tb * 128:(tb + 1) * 128],
                       WO[hp][:, half * 512:(half + 1) * 512],
                       start=(hp == 0), stop=(hp == HP - 1))
        for ti in range(4):
            tb = tbg * 4 + ti
            s_ = [stat_pool.tile([128, 1], F32, tag=f"s{half}", name=f"s{half}")
                  for half in range(2)]
            q_ = [stat_pool.tile([128, 1], F32, tag=f"q{half}", name=f"q{half}")
                  for half in range(2)]
            scr = stage_pool.tile([128, 512], F32, tag="scr", name="scr")
            for half in range(2):
                nc.vector.reduce_sum(s_[half][:], zps[(ti, half)][:],
                                     axis=mybir.AxisListType.X)
                nc.scalar.activation(scr[:], zps[(ti, half)][:], A.Square,
                                     accum_out=q_[half][:])
            mean = stat_pool.tile([128, 1], F32, tag="mean", name="mean")
            nc.vector.tensor_add(mean[:], s_[0][:], s_[1][:])
            nc.vector.tensor_scalar_mul(mean[:], mean[:], 1.0 / DIM)
            msq = stat_pool.tile([128, 1], F32, tag="msq", name="msq")
            nc.vector.tensor_add(msq[:], q_[0][:], q_[1][:])
            nc.vector.tensor_scalar_mul(msq[:], msq[:], 1.0 / DIM)
            var = stat_pool.tile([128, 1], F32, tag="var", name="var")
            nc.vector.tensor_mul(var[:], mean[:], mean[:])
            nc.vector.tensor_sub(var[:], msq[:], var[:])
            std = stat_pool.tile([128, 1], F32, tag="std", name="std")
            nc.scalar.activation(std[:], var[:], A.Sqrt, bias=epst[:])
            rstd = stat_pool.tile([128, 1], F32, tag="rstd", name="rstd")
            nc.vector.reciprocal(rstd[:], std[:])
            nmr = stat_pool.tile([128, 1], F32, tag="nmr", name="nmr")
            nc.vector.tensor_mul(nmr[:], mean[:], rstd[:])
            nc.vector.tensor_scalar_mul(nmr[:], nmr[:], -1.0)
            outb = stage_pool.tile([128, DIM], F32, tag="outb", name="outb")
            for half in range(2):
                zn = stage_pool.tile([128, 512], F32, tag=f"zn{half}",
                                     name=f"zn{half}")
                nc.scalar.activation(zn[:], zps[(ti, half)][:], A.Identity,
                                     bias=nmr[:], scale=rstd[:])
                nc.vector.tensor_mul(outb[:, half * 512:(half + 1) * 512],
                                     zn[:], GB[:, half * 512:(half + 1) * 512])
            nc.gpsimd.dma_start(out[tb * 128:(tb + 1) * 128, :], outb[:])

    z_ps.release()
    stage_pool.release()
    stat_pool.release()
    otz_pool.release()
    wo_pool.release()
    dram.release()
    vpool.release()
    res.release()


def make_in_maps(x, mask, w_qkv, w_out, ln_g):
    x = np.asarray(x, dtype=np.float32)
    mask_np = np.asarray(mask)
    w_qkv = np.asarray(w_qkv, dtype=np.float32)
    w_out = np.ascontiguousarray(np.asarray(w_out, dtype=np.float32))
    ln_g = np.asarray(ln_g, dtype=np.float32)

    wq = np.ascontiguousarray(w_qkv[:, :INNER])
    wk = np.ascontiguousarray(w_qkv[:, INNER:2 * INNER])
    wv = np.ascontiguousarray(w_qkv[:, 2 * INNER:])
    lng = np.ascontiguousarray(ln_g[None, :])

    # diagonal-strip causal mask [128 k, 4 j x (block2g 128 | block2g+1 128)]
    kk = np.arange(128)[:, None]
    r = np.arange(128)[None, :]
    dmasks = {}
    for rho in (0, 1):
        m0 = np.where(kk <= 2 * r + rho, 0.0, NEG).astype(np.float32)
        m1 = np.where(kk + 128 <= 2 * r + rho, 0.0, NEG).astype(np.float32)
        z = np.zeros((128, 128), np.float32)
        ng = np.full((128, 128), NEG, np.float32)
        dmasks[rho] = np.ascontiguousarray(
            np.concatenate([m0, z, m1, z, ng, m0, ng, m1], axis=1))

    in_maps = []
    for b in range(B):
        xf = x[b]  # [N, DIM]
        xfT = np.ascontiguousarray(xf.T)
        mv = mask_np[b].astype(np.float32)  # [N]
        mvecT = np.ascontiguousarray(mv.reshape(KC, 128).T)  # [128, KC]
        for rho in (0, 1):
            xqT = np.ascontiguousarray(xf[rho::2, :].T)
            in_maps.append({
                "xfT": xfT, "xqT": xqT, "wq": wq, "wk": wk, "wv": wv,
                "wout": w_out, "lng": lng, "dmask": dmasks[rho],
                "mvecT": mvecT,
            })
    return in_maps


_CACHE = {}
_LOCK = threading.Lock()


def _get_nc():
    with _LOCK:
        if "nc" not in _CACHE:
            _CACHE["nc"] = build()
    return _CACHE["nc"]


def kernel(x, mask, w_qkv, w_out, ln_g):
    in_maps = make_in_maps(x, mask, w_qkv, w_out, ln_g)
    nc = _get_nc()
    res = bass_utils.run_bass_kernel_spmd(nc, in_maps, core_ids=list(range(NC)))

    final = np.empty((B, N, DIM), dtype=np.float32)
    for b in range(B):
        for rho in (0, 1):
            final[b, rho::2, :] = res.results[2 * b + rho]["out"]
    return final
